# revision 11
# baseline (speedup 1.0000x reference)
"""Trainium2 Bass kernel for nn_Attention_13718125543518.

Dense MHA (B=16, N=1024, DIM=512, H=8, DH=64) with additive positional
bias and softmax:  y = softmax(q k^T / sqrt(dh) + pos_bias) v @ Wo^T.

Sharding: data-parallel over batch. Each of the 8 cores processes 2
batches and all 8 heads; no cross-core communication.

Device-side algorithm (per core, all matmul operands fp16, PSUM fp32):
  - host precomputes xT[c,i] (x transposed), EB[h,j,i] = exp(pos_bias[h,i,j]),
    and weight layouts; the 1/sqrt(dh) scale is folded into Wq.
  - qkT_h[:,i] = [Wq_h^T | Wk_h^T]^T . xT  -> [128, 2048] (rows 0:64 = q^T,
    rows 64:128 = k^T), plus a partition-swapped copy kqT (DMA) so both the
    (0,0) and (64,0) PE row-tiles can compute score tiles concurrently.
  - scores s^T[j,i] = sum_d k[j,d] q[i,d] (K=64 matmuls, two per PE pass via
    row tiling), ACT evacuates PSUM with exp(), DVE multiplies by EB
    (factorized softmax: exp(qk+b) = exp(qk)*exp(b); logits are O(6) so no
    max-subtraction is needed).
  - AV: out^T[d,i] = sum_j v'[j,d] P[j,i] with v' = [v_h | 1] (M=65): row 64
    accumulates the softmax denominator Z[i] for free.
  - normalize with reciprocal_approx_fast + DMA partition-broadcast, then
    y[i,f] = sum_h out_h^T . Wo_h^T (K=64 accumulation over heads).
"""

import threading
from contextlib import ExitStack

import numpy as np

import concourse.bacc as bacc
import concourse.bass as bass
import concourse.mybir as mybir
import concourse.tile as tile
from concourse.bass_utils import run_bass_kernel_spmd

B, N, DIM, H, DH = 16, 1024, 512, 8, 64
SCALE = DH**-0.5
NCORES = 8
NB = B // NCORES  # batches per core
F16 = mybir.dt.float16
F32 = mybir.dt.float32

_lock = threading.Lock()
_built = {}


def emit(tc, xt, eb, wqk, wvt, wot, y, nb=NB, h_=H, n=N, dim=DIM):
    """Emit the per-core program. xt:[dim,nb*n] eb:[h,n,n] wqk:[cc,128,h*128]
    wvt:[cc,128,dim] wot:[h,DH,dim] y:[nb,n,dim]."""
    nc = tc.nc
    Exp = mybir.ActivationFunctionType.Exp
    cc_n = dim // 128  # contraction chunks of the input dim
    jt_n = n // 128  # key tiles per sequence
    it_n = n // 128  # output row tiles per sequence
    i_tot = nb * n  # tokens handled by this core
    nblk = i_tot // 512  # qk-projection column blocks
    dv = DH + 1  # v plus the ones column
    ev = h_ * DH  # total v width across heads

    with ExitStack() as ctx:

        def pool(name, bufs):
            return ctx.enter_context(tc.tile_pool(name=name, bufs=bufs))

        xt_p = pool("xt", cc_n)
        wqk_p = pool("wqk", cc_n)
        wvt_p = pool("wvt", cc_n)
        wot_p = pool("wot", h_)
        v_p = pool("v", i_tot // 128)
        qk_p = pool("qk", 2)
        kq_p = pool("kq", 2)
        e_p = pool("e", min(2 * jt_n, jt_n + 4))
        p_p = pool("p", jt_n + 2)
        eq_p = pool("eq", 3)
        o_p = pool("o", nb * h_)
        z_p = pool("z", 2)
        zsp_p = pool("zsp", 2)
        zb_p = pool("zb", 2)
        y_p = pool("y", 2)
        psA = ctx.enter_context(
            tc.tile_pool(name="psA", bufs=2, space=bass.MemorySpace.PSUM)
        )
        psV = ctx.enter_context(
            tc.tile_pool(name="psV", bufs=2, space=bass.MemorySpace.PSUM)
        )

        # ---- persistent loads ----
        xt_sb = []
        for cc in range(cc_n):
            t = xt_p.tile([128, i_tot], F16, tag="xt")
            nc.sync.dma_start(t[:], xt[cc * 128 : (cc + 1) * 128, :])
            xt_sb.append(t)
        wqk_sb = []
        for cc in range(cc_n):
            t = wqk_p.tile([128, h_ * 128], F16, tag="wqk")
            nc.sync.dma_start(t[:], wqk[cc, :, :])
            wqk_sb.append(t)
        wvt_sb = []
        for cc in range(cc_n):
            t = wvt_p.tile([128, ev], F16, tag="wvt")
            nc.sync.dma_start(t[:], wvt[cc, :, :])
            wvt_sb.append(t)
        wot_sb = []
        for hh in range(h_):
            t = wot_p.tile([DH, dim], F16, tag="wot")
            nc.sync.dma_start(t[:], wot[hh, :, :])
            wot_sb.append(t)

        # ---- V projection (all heads), with ones column ----
        v_sb = []
        for jtg in range(i_tot // 128):
            vt = v_p.tile([128, h_ * dv], F16, tag="v")
            vt3 = vt[:].rearrange("p (h e) -> p h e", h=h_)
            ps = psA.tile([128, ev], F32, tag="psA")
            for cc in range(cc_n):
                nc.tensor.matmul(
                    ps[:],
                    xt_sb[cc][:, jtg * 128 : (jtg + 1) * 128],
                    wvt_sb[cc][:],
                    start=(cc == 0),
                    stop=(cc == cc_n - 1),
                )
            nc.vector.memset(vt3[:, :, DH : DH + 1], 1.0)
            nc.vector.tensor_copy(
                vt3[:, :, 0:DH], ps[:].rearrange("p (h e) -> p h e", h=h_)
            )
            v_sb.append(vt)

        # ---- per-head attention ----
        o_tiles = {}
        for hh in range(h_):
            e_sb = []
            for jt in range(jt_n):
                et = e_p.tile([128, n], F16, tag="e")
                nc.sync.dma_start(et[:], eb[hh, jt * 128 : (jt + 1) * 128, :])
                e_sb.append(et)

            # q/k projection: rows 0:64 q^T (scaled), rows 64:128 k^T
            qk_t = qk_p.tile([128, i_tot], F16, tag="qk")
            for np_ in range((nblk + 1) // 2):
                w = min(2, nblk - np_ * 2)
                ps = psA.tile([128, 512 * w], F32, tag="psA")
                for cc in range(cc_n):
                    for nh in range(w):
                        nn = np_ * 2 + nh
                        nc.tensor.matmul(
                            ps[:, nh * 512 : (nh + 1) * 512],
                            wqk_sb[cc][:, hh * 128 : (hh + 1) * 128],
                            xt_sb[cc][:, nn * 512 : (nn + 1) * 512],
                            start=(cc == 0),
                            stop=(cc == cc_n - 1),
                        )
                nc.vector.tensor_copy(
                    qk_t[:, np_ * 1024 : np_ * 1024 + 512 * w], ps[:]
                )
            kq_t = kq_p.tile([128, i_tot], F16, tag="kq")
            nc.sync.dma_start(kq_t[0:64, :], qk_t[64:128, :])
            nc.sync.dma_start(kq_t[64:128, :], qk_t[0:64, :])

            for b in range(nb):
                # scores -> exp -> *EB, producing P[jt] [128, n] fp16
                p_sb = []
                for jp in range(jt_n // 2):
                    pA = p_p.tile([128, n], F16, tag="p")
                    pB = p_p.tile([128, n], F16, tag="p")
                    jA, jB = 2 * jp, 2 * jp + 1
                    for ib in range(n // 512):
                        ii = b * n + ib * 512
                        jjA = b * n + jA * 128
                        jjB = b * n + jB * 128
                        ps = psA.tile([128, 1024], F32, tag="psA")
                        nc.tensor.matmul(
                            ps[:, 0:512],
                            kq_t[0:64, jjA : jjA + 128],
                            qk_t[0:64, ii : ii + 512],
                        )
                        nc.tensor.matmul(
                            ps[:, 512:1024],
                            qk_t[64:128, jjB : jjB + 128],
                            kq_t[64:128, ii : ii + 512],
                        )
                        eq = eq_p.tile([128, 1024], F16, tag="eq")
                        nc.scalar.activation(eq[:], ps[:], Exp)
                        nc.vector.tensor_mul(
                            pA[:, ib * 512 : ib * 512 + 512],
                            eq[:, 0:512],
                            e_sb[jA][:, ib * 512 : ib * 512 + 512],
                        )
                        nc.vector.tensor_mul(
                            pB[:, ib * 512 : ib * 512 + 512],
                            eq[:, 512:1024],
                            e_sb[jB][:, ib * 512 : ib * 512 + 512],
                        )
                    p_sb += [pA, pB]

                # AV with ones column: row 64 of PSUM accumulates Z
                pso = psV.tile([dv, n], F32, tag="psV")
                for ib in range(n // 512):
                    for jt in range(jt_n):
                        nc.tensor.matmul(
                            pso[:, ib * 512 : (ib + 1) * 512],
                            v_sb[b * jt_n + jt][:, hh * dv : (hh + 1) * dv],
                            p_sb[jt][:, ib * 512 : (ib + 1) * 512],
                            start=(jt == 0),
                            stop=(jt == jt_n - 1),
                        )

                # normalize: out^T[d,i] * (1/Z[i]).  Z sits on PSUM partition
                # 64; spread it across partitions by DMA so the exact DVE
                # reciprocal runs on [128, n/128] (cheap), then broadcast to
                # 64 partitions with doubling DMAs.
                zr = z_p.tile([65, n], F32, tag="zr")
                nc.scalar.copy(zr[64:65, :], pso[DH : DH + 1, :])
                zsp = zsp_p.tile([128, n // 128], F32, tag="zsp")
                nc.sync.dma_start(zsp[:], zr[64:65, :])
                nc.vector.reciprocal(zsp[:], zsp[:])
                zb = zb_p.tile([DH, n], F32, tag="zb")
                nc.sync.dma_start(zb[0:1, :], zsp[:])
                w = 1
                while w < DH:
                    nc.sync.dma_start(zb[w : 2 * w, :], zb[0:w, :])
                    w *= 2
                ot = o_p.tile([DH, n], F16, tag="o")
                nc.vector.tensor_mul(ot[:], pso[0:DH, :], zb[:])
                o_tiles[(b, hh)] = ot

        # ---- output projection: y[i,f] = sum_h out_h^T . WoT_h ----
        for b in range(nb):
            for it in range(it_n):
                psy = psA.tile([128, dim], F32, tag="psA")
                for hh in range(h_):
                    nc.tensor.matmul(
                        psy[:],
                        o_tiles[(b, hh)][:, it * 128 : (it + 1) * 128],
                        wot_sb[hh][:],
                        start=(hh == 0),
                        stop=(hh == h_ - 1),
                    )
                yt = y_p.tile([128, dim], F32, tag="y")
                nc.vector.tensor_copy(yt[:], psy[:])
                nc.sync.dma_start(y[b, it * 128 : (it + 1) * 128, :], yt[:])


def build(nb=NB, h_=H, n=N, dim=DIM):
    nc = bacc.Bacc("TRN2", target_bir_lowering=False, debug=False)
    cc_n = dim // 128
    i_tot = nb * n
    xt_d = nc.dram_tensor("xt", [dim, i_tot], F16, kind="ExternalInput")
    eb_d = nc.dram_tensor("eb", [h_, n, n], F16, kind="ExternalInput")
    wqk_d = nc.dram_tensor("wqk", [cc_n, 128, h_ * 128], F16, kind="ExternalInput")
    wvt_d = nc.dram_tensor("wvt", [cc_n, 128, h_ * DH], F16, kind="ExternalInput")
    wot_d = nc.dram_tensor("wot", [h_, DH, dim], F16, kind="ExternalInput")
    y_d = nc.dram_tensor("y", [nb, n, dim], F32, kind="ExternalOutput")
    with tile.TileContext(nc) as tc:
        emit(
            tc,
            xt_d.ap(),
            eb_d.ap(),
            wqk_d.ap(),
            wvt_d.ap(),
            wot_d.ap(),
            y_d.ap(),
            nb=nb,
            h_=h_,
            n=n,
            dim=dim,
        )
    nc.compile()
    return nc


def prep_inputs(x, pos_bias, Wq, Wk, Wv, Wo, nb=NB, h_=H, n=N, dim=DIM):
    """Host-side layout prep. Returns per-core input maps."""
    x = np.asarray(x, np.float32)
    pos_bias = np.asarray(pos_bias, np.float32)
    b_tot = x.shape[0]
    ncores = b_tot // nb
    cc_n = dim // 128
    dh = DH

    xT = np.ascontiguousarray(x.reshape(b_tot * n, dim).T).astype(np.float16)
    ebt = np.ascontiguousarray(np.exp(pos_bias).transpose(0, 2, 1)).astype(np.float16)

    wqs = np.asarray(Wq, np.float32).T * SCALE  # [c, e]
    wkt = np.asarray(Wk, np.float32).T
    wqk = np.empty([cc_n, 128, h_ * 128], np.float16)
    for cc in range(cc_n):
        for hh in range(h_):
            cs = slice(cc * 128, (cc + 1) * 128)
            es = slice(hh * dh, (hh + 1) * dh)
            wqk[cc, :, hh * 128 : hh * 128 + dh] = wqs[cs, es]
            wqk[cc, :, hh * 128 + dh : hh * 128 + 2 * dh] = wkt[cs, es]
    wvt = np.ascontiguousarray(np.asarray(Wv, np.float32).T).astype(np.float16)
    wvt = wvt.reshape(cc_n, 128, h_ * dh)
    wot = np.ascontiguousarray(np.asarray(Wo, np.float32).T).astype(np.float16)
    wot = wot.reshape(h_, dh, dim)

    i_tot = nb * n
    in_maps = []
    for c in range(ncores):
        in_maps.append(
            {
                "xt": np.ascontiguousarray(xT[:, c * i_tot : (c + 1) * i_tot]),
                "eb": ebt,
                "wqk": wqk,
                "wvt": wvt,
                "wot": wot,
            }
        )
    return in_maps


def get_built():
    with _lock:
        if "nc" not in _built:
            _built["nc"] = build()
        return _built["nc"]


def run_on_device(in_maps, **kwargs):
    nc = get_built()
    return run_bass_kernel_spmd(nc, in_maps, core_ids=list(range(len(in_maps))), **kwargs)


def kernel(x, pos_bias, Wq, Wk, Wv, Wo):
    in_maps = prep_inputs(x, pos_bias, Wq, Wk, Wv, Wo)
    res = run_on_device(in_maps)
    y = np.concatenate([r["y"] for r in res.results], axis=0)
    return np.ascontiguousarray(y.astype(np.float32))


# revision 13
# speedup vs baseline: 1.0936x; 1.0936x over previous
"""Trainium2 Bass kernel for nn_Attention_13718125543518.

Dense MHA (B=16, N=1024, DIM=512, H=8, DH=64) with additive positional
bias and softmax:  y = softmax(q k^T / sqrt(dh) + pos_bias) v @ Wo^T.

Sharding: data-parallel over batch. Each of the 8 cores processes 2
batches and all 8 heads; no cross-core communication.

Device-side algorithm (per core, all matmul operands fp16, PSUM fp32):
  - host precomputes xT[c,i] (x transposed), EB[h,j,i] = exp(pos_bias[h,i,j]),
    and weight layouts; the 1/sqrt(dh) scale is folded into Wq.
  - qkT_h[:,i] = [Wq_h^T | Wk_h^T]^T . xT  -> [128, 2048] (rows 0:64 = q^T,
    rows 64:128 = k^T), plus a partition-swapped copy kqT (DMA) so both the
    (0,0) and (64,0) PE row-tiles can compute score tiles concurrently.
  - scores s^T[j,i] = sum_d k[j,d] q[i,d] (K=64 matmuls, two per PE pass via
    row tiling), ACT evacuates PSUM with exp(), DVE multiplies by EB
    (factorized softmax: exp(qk+b) = exp(qk)*exp(b); logits are O(6) so no
    max-subtraction is needed).
  - AV: out^T[d,i] = sum_j v'[j,d] P[j,i] with v' = [v_h | 1] (M=65): row 64
    accumulates the softmax denominator Z[i] for free.
  - normalize with reciprocal_approx_fast + DMA partition-broadcast, then
    y[i,f] = sum_h out_h^T . Wo_h^T (K=64 accumulation over heads).
"""

import threading
from contextlib import ExitStack

import numpy as np

import concourse.bacc as bacc
import concourse.bass as bass
import concourse.mybir as mybir
import concourse.tile as tile
from concourse.bass_utils import run_bass_kernel_spmd

B, N, DIM, H, DH = 16, 1024, 512, 8, 64
SCALE = DH**-0.5
NCORES = 8
NB = B // NCORES  # batches per core
F16 = mybir.dt.float16
F32 = mybir.dt.float32

_lock = threading.Lock()
_built = {}


def emit(tc, xt, eb, wqk, wvt, wot, y, nb=NB, h_=H, n=N, dim=DIM):
    """Emit the per-core program. xt:[dim,nb*n] eb:[h,n,n] wqk:[cc,128,h*128]
    wvt:[cc,128,dim] wot:[h,DH,dim] y:[nb,n,dim]."""
    nc = tc.nc
    Exp = mybir.ActivationFunctionType.Exp
    cc_n = dim // 128  # contraction chunks of the input dim
    jt_n = n // 128  # key tiles per sequence
    it_n = n // 128  # output row tiles per sequence
    i_tot = nb * n  # tokens handled by this core
    nblk = i_tot // 512  # qk-projection column blocks
    dv = DH + 1  # v plus the ones column
    ev = h_ * DH  # total v width across heads

    with ExitStack() as ctx:

        def pool(name, bufs):
            return ctx.enter_context(tc.tile_pool(name=name, bufs=bufs))

        xt_p = pool("xt", cc_n)
        wqk_p = pool("wqk", cc_n)
        wvt_p = pool("wvt", cc_n)
        wot_p = pool("wot", h_)
        v_p = pool("v", i_tot // 128)
        qk_p = pool("qk", 2)
        kq_p = pool("kq", 2)
        e_p = pool("e", 2 * jt_n)
        p_p = pool("p", jt_n + 2)
        eq_p = pool("eq", 3)
        o_p = pool("o", nb * h_)
        z_p = pool("z", 2)
        zsp_p = pool("zsp", 2)
        zb_p = pool("zb", 2)
        y_p = pool("y", 2)
        psA = ctx.enter_context(
            tc.tile_pool(name="psA", bufs=2, space=bass.MemorySpace.PSUM)
        )
        psV = ctx.enter_context(
            tc.tile_pool(name="psV", bufs=2, space=bass.MemorySpace.PSUM)
        )

        # ---- persistent loads ----
        xt_sb = []
        for cc in range(cc_n):
            t = xt_p.tile([128, i_tot], F16, tag="xt")
            nc.sync.dma_start(t[:], xt[cc * 128 : (cc + 1) * 128, :])
            xt_sb.append(t)
        wqk_sb = []
        for cc in range(cc_n):
            t = wqk_p.tile([128, h_ * 128], F16, tag="wqk")
            nc.sync.dma_start(t[:], wqk[cc, :, :])
            wqk_sb.append(t)
        wvt_sb = []
        for cc in range(cc_n):
            t = wvt_p.tile([128, ev], F16, tag="wvt")
            nc.sync.dma_start(t[:], wvt[cc, :, :])
            wvt_sb.append(t)
        wot_sb = []
        for hh in range(h_):
            t = wot_p.tile([DH, dim], F16, tag="wot")
            nc.sync.dma_start(t[:], wot[hh, :, :])
            wot_sb.append(t)

        # ---- V projection (all heads), with ones column ----
        v_sb = []
        for jtg in range(i_tot // 128):
            vt = v_p.tile([128, h_ * dv], F16, tag="v")
            vt3 = vt[:].rearrange("p (h e) -> p h e", h=h_)
            ps = psA.tile([128, ev], F32, tag="psA")
            for cc in range(cc_n):
                nc.tensor.matmul(
                    ps[:],
                    xt_sb[cc][:, jtg * 128 : (jtg + 1) * 128],
                    wvt_sb[cc][:],
                    start=(cc == 0),
                    stop=(cc == cc_n - 1),
                )
            nc.vector.memset(vt3[:, :, DH : DH + 1], 1.0)
            nc.vector.tensor_copy(
                vt3[:, :, 0:DH], ps[:].rearrange("p (h e) -> p h e", h=h_)
            )
            v_sb.append(vt)

        # ---- per-head attention, software-pipelined: head h+1's bias
        # loads, q/k projection and swap are emitted before head h's
        # attention so the PE never waits on the evac+swap chain.
        def head_prep(hh):
            e_sb = []
            for jt in range(jt_n):
                et = e_p.tile([128, n], F16, tag="e")
                nc.sync.dma_start(et[:], eb[hh, jt * 128 : (jt + 1) * 128, :])
                e_sb.append(et)

            # q/k projection: rows 0:64 q^T (scaled), rows 64:128 k^T
            qk_t = qk_p.tile([128, i_tot], F16, tag="qk")
            for np_ in range((nblk + 1) // 2):
                w = min(2, nblk - np_ * 2)
                ps = psA.tile([128, 512 * w], F32, tag="psA")
                for cc in range(cc_n):
                    for nh in range(w):
                        nn = np_ * 2 + nh
                        nc.tensor.matmul(
                            ps[:, nh * 512 : (nh + 1) * 512],
                            wqk_sb[cc][:, hh * 128 : (hh + 1) * 128],
                            xt_sb[cc][:, nn * 512 : (nn + 1) * 512],
                            start=(cc == 0),
                            stop=(cc == cc_n - 1),
                        )
                nc.vector.tensor_copy(
                    qk_t[:, np_ * 1024 : np_ * 1024 + 512 * w], ps[:]
                )
            kq_t = kq_p.tile([128, i_tot], F16, tag="kq")
            nc.sync.dma_start(kq_t[0:64, :], qk_t[64:128, :])
            nc.sync.dma_start(kq_t[64:128, :], qk_t[0:64, :])
            return e_sb, qk_t, kq_t

        o_tiles = {}
        prep = head_prep(0)
        for hh in range(h_):
            e_sb, qk_t, kq_t = prep
            if hh + 1 < h_:
                prep = head_prep(hh + 1)

            for b in range(nb):
                # scores -> exp -> *EB, producing P[jt] [128, n] fp16
                p_sb = []
                for jp in range(jt_n // 2):
                    pA = p_p.tile([128, n], F16, tag="p")
                    pB = p_p.tile([128, n], F16, tag="p")
                    jA, jB = 2 * jp, 2 * jp + 1
                    for ib in range(n // 512):
                        ii = b * n + ib * 512
                        jjA = b * n + jA * 128
                        jjB = b * n + jB * 128
                        ps = psA.tile([128, 1024], F32, tag="psA")
                        nc.tensor.matmul(
                            ps[:, 0:512],
                            kq_t[0:64, jjA : jjA + 128],
                            qk_t[0:64, ii : ii + 512],
                        )
                        nc.tensor.matmul(
                            ps[:, 512:1024],
                            qk_t[64:128, jjB : jjB + 128],
                            kq_t[64:128, ii : ii + 512],
                        )
                        eq = eq_p.tile([128, 1024], F16, tag="eq")
                        nc.scalar.activation(eq[:], ps[:], Exp)
                        nc.vector.tensor_mul(
                            pA[:, ib * 512 : ib * 512 + 512],
                            eq[:, 0:512],
                            e_sb[jA][:, ib * 512 : ib * 512 + 512],
                        )
                        nc.vector.tensor_mul(
                            pB[:, ib * 512 : ib * 512 + 512],
                            eq[:, 512:1024],
                            e_sb[jB][:, ib * 512 : ib * 512 + 512],
                        )
                    p_sb += [pA, pB]

                # AV with ones column: row 64 of PSUM accumulates Z
                pso = psV.tile([dv, n], F32, tag="psV")
                for ib in range(n // 512):
                    for jt in range(jt_n):
                        nc.tensor.matmul(
                            pso[:, ib * 512 : (ib + 1) * 512],
                            v_sb[b * jt_n + jt][:, hh * dv : (hh + 1) * dv],
                            p_sb[jt][:, ib * 512 : (ib + 1) * 512],
                            start=(jt == 0),
                            stop=(jt == jt_n - 1),
                        )

                # normalize: out^T[d,i] * (1/Z[i]).  Z sits on PSUM partition
                # 64; spread it across partitions by DMA so the exact DVE
                # reciprocal runs on [128, n/128] (cheap), then broadcast to
                # 64 partitions with doubling DMAs.
                zr = z_p.tile([65, n], F32, tag="zr")
                nc.scalar.copy(zr[64:65, :], pso[DH : DH + 1, :])
                zsp = zsp_p.tile([128, n // 128], F32, tag="zsp")
                nc.sync.dma_start(zsp[:], zr[64:65, :])
                nc.vector.reciprocal(zsp[:], zsp[:])
                zb = zb_p.tile([DH, n], F32, tag="zb")
                nc.sync.dma_start(zb[0:1, :], zsp[:])
                w = 1
                while w < DH:
                    nc.sync.dma_start(zb[w : 2 * w, :], zb[0:w, :])
                    w *= 2
                ot = o_p.tile([DH, n], F16, tag="o")
                nc.vector.tensor_mul(ot[:], pso[0:DH, :], zb[:])
                o_tiles[(b, hh)] = ot

        # ---- output projection: y[i,f] = sum_h out_h^T . WoT_h ----
        for b in range(nb):
            for it in range(it_n):
                psy = psA.tile([128, dim], F32, tag="psA")
                for hh in range(h_):
                    nc.tensor.matmul(
                        psy[:],
                        o_tiles[(b, hh)][:, it * 128 : (it + 1) * 128],
                        wot_sb[hh][:],
                        start=(hh == 0),
                        stop=(hh == h_ - 1),
                    )
                yt = y_p.tile([128, dim], F32, tag="y")
                nc.vector.tensor_copy(yt[:], psy[:])
                nc.sync.dma_start(y[b, it * 128 : (it + 1) * 128, :], yt[:])


def build(nb=NB, h_=H, n=N, dim=DIM):
    nc = bacc.Bacc("TRN2", target_bir_lowering=False, debug=False)
    cc_n = dim // 128
    i_tot = nb * n
    xt_d = nc.dram_tensor("xt", [dim, i_tot], F16, kind="ExternalInput")
    eb_d = nc.dram_tensor("eb", [h_, n, n], F16, kind="ExternalInput")
    wqk_d = nc.dram_tensor("wqk", [cc_n, 128, h_ * 128], F16, kind="ExternalInput")
    wvt_d = nc.dram_tensor("wvt", [cc_n, 128, h_ * DH], F16, kind="ExternalInput")
    wot_d = nc.dram_tensor("wot", [h_, DH, dim], F16, kind="ExternalInput")
    y_d = nc.dram_tensor("y", [nb, n, dim], F32, kind="ExternalOutput")
    with tile.TileContext(nc) as tc:
        emit(
            tc,
            xt_d.ap(),
            eb_d.ap(),
            wqk_d.ap(),
            wvt_d.ap(),
            wot_d.ap(),
            y_d.ap(),
            nb=nb,
            h_=h_,
            n=n,
            dim=dim,
        )
    nc.compile()
    return nc


def prep_inputs(x, pos_bias, Wq, Wk, Wv, Wo, nb=NB, h_=H, n=N, dim=DIM):
    """Host-side layout prep. Returns per-core input maps."""
    x = np.asarray(x, np.float32)
    pos_bias = np.asarray(pos_bias, np.float32)
    b_tot = x.shape[0]
    ncores = b_tot // nb
    cc_n = dim // 128
    dh = DH

    xT = np.ascontiguousarray(x.reshape(b_tot * n, dim).T).astype(np.float16)
    ebt = np.ascontiguousarray(np.exp(pos_bias).transpose(0, 2, 1)).astype(np.float16)

    wqs = np.asarray(Wq, np.float32).T * SCALE  # [c, e]
    wkt = np.asarray(Wk, np.float32).T
    wqk = np.empty([cc_n, 128, h_ * 128], np.float16)
    for cc in range(cc_n):
        for hh in range(h_):
            cs = slice(cc * 128, (cc + 1) * 128)
            es = slice(hh * dh, (hh + 1) * dh)
            wqk[cc, :, hh * 128 : hh * 128 + dh] = wqs[cs, es]
            wqk[cc, :, hh * 128 + dh : hh * 128 + 2 * dh] = wkt[cs, es]
    wvt = np.ascontiguousarray(np.asarray(Wv, np.float32).T).astype(np.float16)
    wvt = wvt.reshape(cc_n, 128, h_ * dh)
    wot = np.ascontiguousarray(np.asarray(Wo, np.float32).T).astype(np.float16)
    wot = wot.reshape(h_, dh, dim)

    i_tot = nb * n
    in_maps = []
    for c in range(ncores):
        in_maps.append(
            {
                "xt": np.ascontiguousarray(xT[:, c * i_tot : (c + 1) * i_tot]),
                "eb": ebt,
                "wqk": wqk,
                "wvt": wvt,
                "wot": wot,
            }
        )
    return in_maps


def get_built():
    with _lock:
        if "nc" not in _built:
            _built["nc"] = build()
        return _built["nc"]


def run_on_device(in_maps, **kwargs):
    nc = get_built()
    return run_bass_kernel_spmd(nc, in_maps, core_ids=list(range(len(in_maps))), **kwargs)


def kernel(x, pos_bias, Wq, Wk, Wv, Wo):
    in_maps = prep_inputs(x, pos_bias, Wq, Wk, Wv, Wo)
    res = run_on_device(in_maps)
    y = np.concatenate([r["y"] for r in res.results], axis=0)
    return np.ascontiguousarray(y.astype(np.float32))


# revision 16
# speedup vs baseline: 1.5175x; 1.3876x over previous
"""Trainium2 Bass kernel for nn_Attention_13718125543518.

Dense MHA (B=16, N=1024, DIM=512, H=8, DH=64) with additive positional
bias and softmax:  y = softmax(q k^T / sqrt(dh) + pos_bias) v @ Wo^T.

Sharding: data-parallel over batch. Each of the 8 cores processes 2
batches and all 8 heads; no cross-core communication.

Device-side algorithm (per core, all matmul operands fp16, PSUM fp32):
  - host precomputes xT[c,i] (x transposed), EB[h,j,i] = exp(pos_bias[h,i,j]),
    and weight layouts; the 1/sqrt(dh) scale is folded into Wq.
  - qkT_h[:,i] = [Wq_h^T | Wk_h^T]^T . xT  -> [128, 2048] (rows 0:64 = q^T,
    rows 64:128 = k^T), plus a partition-swapped copy kqT (DMA) so both the
    (0,0) and (64,0) PE row-tiles can compute score tiles concurrently.
  - scores s^T[j,i] = sum_d k[j,d] q[i,d] (K=64 matmuls, two per PE pass via
    row tiling), ACT evacuates PSUM with exp(), DVE multiplies by EB
    (factorized softmax: exp(qk+b) = exp(qk)*exp(b); logits are O(6) so no
    max-subtraction is needed).
  - AV: out^T[d,i] = sum_j v'[j,d] P[j,i] with v' = [v_h | 1] (M=65): row 64
    accumulates the softmax denominator Z[i] for free.
  - normalize with reciprocal_approx_fast + DMA partition-broadcast, then
    y[i,f] = sum_h out_h^T . Wo_h^T (K=64 accumulation over heads).
"""

import threading
from contextlib import ExitStack

import numpy as np

import concourse.bacc as bacc
import concourse.bass as bass
import concourse.mybir as mybir
import concourse.tile as tile
from concourse.bass_utils import run_bass_kernel_spmd

B, N, DIM, H, DH = 16, 1024, 512, 8, 64
SCALE = DH**-0.5
NCORES = 8
NB = B // NCORES  # batches per core
F16 = mybir.dt.float16
F32 = mybir.dt.float32

_lock = threading.Lock()
_built = {}


def emit(tc, xt, eb, wqk, wvt, wot, y, nb=NB, h_=H, n=N, dim=DIM):
    """Emit the per-core program. xt:[dim,nb*n] eb:[h,n,n] wqk:[cc,128,h*128]
    wvt:[cc,128,dim] wot:[h,DH,dim] y:[nb,n,dim]."""
    nc = tc.nc
    Exp = mybir.ActivationFunctionType.Exp
    cc_n = dim // 128  # contraction chunks of the input dim
    jt_n = n // 128  # key tiles per sequence
    it_n = n // 128  # output row tiles per sequence
    i_tot = nb * n  # tokens handled by this core
    nblk = i_tot // 512  # qk-projection column blocks
    dv = DH + 1  # v plus the ones column
    ev = h_ * DH  # total v width across heads

    with ExitStack() as ctx:

        def pool(name, bufs):
            return ctx.enter_context(tc.tile_pool(name=name, bufs=bufs))

        xt_p = pool("xt", cc_n)
        wqk_p = pool("wqk", cc_n)
        wvt_p = pool("wvt", cc_n)
        wot_p = pool("wot", h_)
        v_p = pool("v", i_tot // 128)
        qk_p = pool("qk", 2)
        kq_p = pool("kq", 2)
        e_p = pool("e", 2 * jt_n)
        p_p = pool("p", jt_n + 2)
        eq_p = pool("eq", 3)
        o_p = pool("o", nb * h_)
        raw_p = pool("raw", 4)
        zsp_p = pool("zsp", 3)
        zb_p = pool("zb", 3)
        o2_p = pool("o2", nb * h_ // 2)
        y_p = pool("y", 2)
        psA = ctx.enter_context(
            tc.tile_pool(name="psA", bufs=2, space=bass.MemorySpace.PSUM)
        )
        psV = ctx.enter_context(
            tc.tile_pool(name="psV", bufs=2, space=bass.MemorySpace.PSUM)
        )

        # ---- persistent loads ----
        xt_sb = []
        for cc in range(cc_n):
            t = xt_p.tile([128, i_tot], F16, tag="xt")
            nc.sync.dma_start(t[:], xt[cc * 128 : (cc + 1) * 128, :])
            xt_sb.append(t)
        wqk_sb = []
        for cc in range(cc_n):
            t = wqk_p.tile([128, h_ * 128], F16, tag="wqk")
            nc.sync.dma_start(t[:], wqk[cc, :, :])
            wqk_sb.append(t)
        wvt_sb = []
        for cc in range(cc_n):
            t = wvt_p.tile([128, ev], F16, tag="wvt")
            nc.sync.dma_start(t[:], wvt[cc, :, :])
            wvt_sb.append(t)
        # Wo chunks for head PAIRS: [128, dim] (K=128 in the out-projection)
        wot_sb = []
        for hp in range(h_ // 2):
            t = wot_p.tile([128, dim], F16, tag="wot")
            nc.sync.dma_start(t[:], wot[2 * hp : 2 * hp + 2, :, :])
            wot_sb.append(t)

        # ---- V projection (all heads), with ones column ----
        v_sb = []
        for jtg in range(i_tot // 128):
            vt = v_p.tile([128, h_ * dv], F16, tag="v")
            vt3 = vt[:].rearrange("p (h e) -> p h e", h=h_)
            ps = psA.tile([128, ev], F32, tag="psA")
            for cc in range(cc_n):
                nc.tensor.matmul(
                    ps[:],
                    xt_sb[cc][:, jtg * 128 : (jtg + 1) * 128],
                    wvt_sb[cc][:],
                    start=(cc == 0),
                    stop=(cc == cc_n - 1),
                )
            nc.vector.memset(vt3[:, :, DH : DH + 1], 1.0)
            nc.vector.tensor_copy(
                vt3[:, :, 0:DH], ps[:].rearrange("p (h e) -> p h e", h=h_)
            )
            v_sb.append(vt)

        # ---- per-head attention, software-pipelined: head h+1's bias
        # loads, q/k projection and swap are emitted before head h's
        # attention so the PE never waits on the evac+swap chain.
        def head_prep(hh):
            e_sb = []
            for jt in range(jt_n):
                et = e_p.tile([128, n], F16, tag="e")
                nc.sync.dma_start(et[:], eb[hh, jt * 128 : (jt + 1) * 128, :])
                e_sb.append(et)

            # q/k projection: rows 0:64 q^T (scaled), rows 64:128 k^T
            qk_t = qk_p.tile([128, i_tot], F16, tag="qk")
            for np_ in range((nblk + 1) // 2):
                w = min(2, nblk - np_ * 2)
                ps = psA.tile([128, 512 * w], F32, tag="psA")
                for cc in range(cc_n):
                    for nh in range(w):
                        nn = np_ * 2 + nh
                        nc.tensor.matmul(
                            ps[:, nh * 512 : (nh + 1) * 512],
                            wqk_sb[cc][:, hh * 128 : (hh + 1) * 128],
                            xt_sb[cc][:, nn * 512 : (nn + 1) * 512],
                            start=(cc == 0),
                            stop=(cc == cc_n - 1),
                        )
                nc.vector.tensor_copy(
                    qk_t[:, np_ * 1024 : np_ * 1024 + 512 * w], ps[:]
                )
            kq_t = kq_p.tile([128, i_tot], F16, tag="kq")
            nc.sync.dma_start(kq_t[0:64, :], qk_t[64:128, :])
            nc.sync.dma_start(kq_t[64:128, :], qk_t[0:64, :])
            return e_sb, qk_t, kq_t

        # Normalize is pipelined 2 steps behind AV: step n copies PSUM out
        # (releasing the accumulator) and spreads Z by DMA; step n+1 runs the
        # reciprocal and launches the broadcast DMA chain; step n+2 does the
        # multiply (broadcast long complete -> no DVE stall).
        o_tiles = {}
        stage1 = []  # (key, raw, zsp)
        stage2 = []  # (key, raw, zb)

        def norm_stage1():
            key, raw, zsp = stage1.pop(0)
            with nc.allow_low_precision("softmax denominator in fp16"):
                nc.vector.reciprocal(zsp[:], zsp[:])
            zb = zb_p.tile([DH, n], F16, tag="zb")
            nc.sync.dma_start(zb[0:1, :], zsp[:])
            w = 1
            while w < DH:
                nc.sync.dma_start(zb[w : 2 * w, :], zb[0:w, :])
                w *= 2
            stage2.append((key, raw, zb))

        def norm_stage2():
            key, raw, zb = stage2.pop(0)
            ot = o_p.tile([DH, n], F16, tag="o")
            nc.vector.tensor_mul(ot[:], raw[0:DH, :], zb[:])
            o_tiles[key] = ot

        prep = head_prep(0)
        for hh in range(h_):
            e_sb, qk_t, kq_t = prep
            if hh + 1 < h_:
                prep = head_prep(hh + 1)

            for b in range(nb):
                # scores -> exp -> *EB, producing P[jt] [128, n] fp16.
                # Weight loads for row-tile pair (A rows 0:64, B rows 64:128)
                # precede both i-blocks so each LDW serves two matmuls.
                p_sb = []
                for jp in range(jt_n // 2):
                    pA = p_p.tile([128, n], F16, tag="p")
                    pB = p_p.tile([128, n], F16, tag="p")
                    jA, jB = 2 * jp, 2 * jp + 1
                    jjA = b * n + jA * 128
                    jjB = b * n + jB * 128
                    pss = []
                    for ib in range(n // 512):
                        ii = b * n + ib * 512
                        ps = psA.tile([128, 1024], F32, tag="psA")
                        nc.tensor.matmul(
                            ps[:, 0:512],
                            kq_t[0:64, jjA : jjA + 128],
                            qk_t[0:64, ii : ii + 512],
                        )
                        nc.tensor.matmul(
                            ps[:, 512:1024],
                            qk_t[64:128, jjB : jjB + 128],
                            kq_t[64:128, ii : ii + 512],
                        )
                        pss.append(ps)
                    for ib in range(n // 512):
                        eq = eq_p.tile([128, 1024], F16, tag="eq")
                        nc.scalar.activation(eq[:], pss[ib][:], Exp)
                        nc.vector.tensor_mul(
                            pA[:, ib * 512 : ib * 512 + 512],
                            eq[:, 0:512],
                            e_sb[jA][:, ib * 512 : ib * 512 + 512],
                        )
                        nc.vector.tensor_mul(
                            pB[:, ib * 512 : ib * 512 + 512],
                            eq[:, 512:1024],
                            e_sb[jB][:, ib * 512 : ib * 512 + 512],
                        )
                    p_sb += [pA, pB]

                if stage2:
                    norm_stage2()

                # AV with ones column: row 64 of PSUM accumulates Z.
                # jt outer so each V weight load serves both i-blocks.
                pso = psV.tile([dv, n], F32, tag="psV")
                for jt in range(jt_n):
                    for ib in range(n // 512):
                        nc.tensor.matmul(
                            pso[:, ib * 512 : (ib + 1) * 512],
                            v_sb[b * jt_n + jt][:, hh * dv : (hh + 1) * dv],
                            p_sb[jt][:, ib * 512 : (ib + 1) * 512],
                            start=(jt == 0),
                            stop=(jt == jt_n - 1),
                        )

                raw = raw_p.tile([dv, n], F16, tag="raw")
                nc.vector.tensor_copy(raw[:], pso[:])
                zsp = zsp_p.tile([128, n // 128], F16, tag="zsp")
                nc.sync.dma_start(zsp[:], raw[64:65, :])
                stage1.append(((b, hh), raw, zsp))
                if len(stage1) > 1:
                    norm_stage1()

        while stage1:
            norm_stage1()
        while stage2:
            norm_stage2()

        # ---- output projection: y[i,f] = sum_h out_h^T . WoT_h ----
        # Repack head pairs into [128, n] tiles so the contraction is K=128.
        o2 = {}
        for b in range(nb):
            for hp in range(h_ // 2):
                t = o2_p.tile([128, n], F16, tag="o2")
                nc.sync.dma_start(t[0:64, :], o_tiles[(b, 2 * hp)][:])
                nc.sync.dma_start(t[64:128, :], o_tiles[(b, 2 * hp + 1)][:])
                o2[(b, hp)] = t
        for b in range(nb):
            for it in range(it_n):
                psy = psA.tile([128, dim], F32, tag="psA")
                for hp in range(h_ // 2):
                    nc.tensor.matmul(
                        psy[:],
                        o2[(b, hp)][:, it * 128 : (it + 1) * 128],
                        wot_sb[hp][:],
                        start=(hp == 0),
                        stop=(hp == h_ // 2 - 1),
                    )
                yt = y_p.tile([128, dim], F32, tag="y")
                nc.vector.tensor_copy(yt[:], psy[:])
                nc.sync.dma_start(y[b, it * 128 : (it + 1) * 128, :], yt[:])


def build(nb=NB, h_=H, n=N, dim=DIM):
    nc = bacc.Bacc("TRN2", target_bir_lowering=False, debug=False)
    cc_n = dim // 128
    i_tot = nb * n
    xt_d = nc.dram_tensor("xt", [dim, i_tot], F16, kind="ExternalInput")
    eb_d = nc.dram_tensor("eb", [h_, n, n], F16, kind="ExternalInput")
    wqk_d = nc.dram_tensor("wqk", [cc_n, 128, h_ * 128], F16, kind="ExternalInput")
    wvt_d = nc.dram_tensor("wvt", [cc_n, 128, h_ * DH], F16, kind="ExternalInput")
    wot_d = nc.dram_tensor("wot", [h_, DH, dim], F16, kind="ExternalInput")
    y_d = nc.dram_tensor("y", [nb, n, dim], F32, kind="ExternalOutput")
    with tile.TileContext(nc) as tc:
        emit(
            tc,
            xt_d.ap(),
            eb_d.ap(),
            wqk_d.ap(),
            wvt_d.ap(),
            wot_d.ap(),
            y_d.ap(),
            nb=nb,
            h_=h_,
            n=n,
            dim=dim,
        )
    nc.compile()
    return nc


def prep_inputs(x, pos_bias, Wq, Wk, Wv, Wo, nb=NB, h_=H, n=N, dim=DIM):
    """Host-side layout prep. Returns per-core input maps."""
    x = np.asarray(x, np.float32)
    pos_bias = np.asarray(pos_bias, np.float32)
    b_tot = x.shape[0]
    ncores = b_tot // nb
    cc_n = dim // 128
    dh = DH

    xT = np.ascontiguousarray(x.reshape(b_tot * n, dim).T).astype(np.float16)
    ebt = np.ascontiguousarray(np.exp(pos_bias).transpose(0, 2, 1)).astype(np.float16)

    wqs = np.asarray(Wq, np.float32).T * SCALE  # [c, e]
    wkt = np.asarray(Wk, np.float32).T
    wqk = np.empty([cc_n, 128, h_ * 128], np.float16)
    for cc in range(cc_n):
        for hh in range(h_):
            cs = slice(cc * 128, (cc + 1) * 128)
            es = slice(hh * dh, (hh + 1) * dh)
            wqk[cc, :, hh * 128 : hh * 128 + dh] = wqs[cs, es]
            wqk[cc, :, hh * 128 + dh : hh * 128 + 2 * dh] = wkt[cs, es]
    wvt = np.ascontiguousarray(np.asarray(Wv, np.float32).T).astype(np.float16)
    wvt = wvt.reshape(cc_n, 128, h_ * dh)
    wot = np.ascontiguousarray(np.asarray(Wo, np.float32).T).astype(np.float16)
    wot = wot.reshape(h_, dh, dim)

    i_tot = nb * n
    in_maps = []
    for c in range(ncores):
        in_maps.append(
            {
                "xt": np.ascontiguousarray(xT[:, c * i_tot : (c + 1) * i_tot]),
                "eb": ebt,
                "wqk": wqk,
                "wvt": wvt,
                "wot": wot,
            }
        )
    return in_maps


def get_built():
    with _lock:
        if "nc" not in _built:
            _built["nc"] = build()
        return _built["nc"]


def run_on_device(in_maps, **kwargs):
    nc = get_built()
    return run_bass_kernel_spmd(nc, in_maps, core_ids=list(range(len(in_maps))), **kwargs)


def kernel(x, pos_bias, Wq, Wk, Wv, Wo):
    in_maps = prep_inputs(x, pos_bias, Wq, Wk, Wv, Wo)
    res = run_on_device(in_maps)
    y = np.concatenate([r["y"] for r in res.results], axis=0)
    return np.ascontiguousarray(y.astype(np.float32))
